# revision 1
# baseline (speedup 1.0000x reference)
"""Bahdanau attention Trainium2 kernel, v2.

Problem shapes (fixed): B=64, T=1024, KS=QS=H=1024, fp32 in/out.
  proj_keys = keys @ W_key                  [B,T,H]
  q         = query @ W_query               [B,1,H]
  scores    = tanh(q + proj_keys) . w_score [B,T]
  alphas    = softmax(mask(scores))         [B,1,T]
  context   = alphas @ values               [B,1,KS]

Sharding: data-parallel over batch across 8 NeuronCores (8 batches/core),
weights replicated.

v2 changes vs v1:
  - all matmul operands bf16 (tolerance 2e-2; enables FWL weight loads)
  - keys transposes moved off the PE onto the DMA xbar transpose engine:
    keys are declared [TB, T, H, 2] bf16 over the same fp32 bytes and the
    hi-16 halves (= truncate-to-bf16) are DMA-transposed straight from
    DRAM into SBUF as keysT tiles
  - softmax without max-subtraction (|scores| <= |w_score|_1 ~ 26, exp is
    safe in fp32); normalization deferred: ctx = (sum exp*v) * (1/sum)
    applied on the [1,H] drain, alphas row normalized independently
  - each batch's PE tail (alpha transposes + ctx matmuls) is emitted
    inside the NEXT batch's m-loop so the softmax ACT/DVE chain overlaps
    dense main-matmul PE work instead of stalling the PE queue

PSUM budget (8 banks): ps_pool 2x[128,1024]=4, sc_psum 2x[1,512]=2,
tl_psum (paT/pcx shared ring) 2.  The q-projection psum borrows a ps_pool
tile before the first batch's m-loop.
"""

import numpy as np

import concourse.bass as bass
import concourse.mybir as mybir
import concourse.tile as tile
from concourse.masks import make_identity

f32 = mybir.dt.float32
bf16 = mybir.dt.bfloat16

P = 128        # partitions
TB = 8         # batches per core
T = 1024       # sequence length
H = 1024       # hidden (= KS = QS)
NC_ = 8        # chunks of 128 along T/H/KS
NH = 512       # matmul moving free-dim (one PSUM bank of fp32)
TAIL_M = 2     # m-index in batch b+1's loop where batch b's tail is emitted

AX = mybir.AxisListType
ALU = mybir.AluOpType
ACT = mybir.ActivationFunctionType


def _split_drain_waits(nc, max_waits: int = 1):
    """walrus CTRL encoding supports a limited number of sem waits per
    instruction; Tile's final drain can carry many.  Hoist extras onto
    preceding single-wait drains."""
    for func in nc.m.functions:
        for blk in func.blocks:
            new_insts = []
            for inst in blk.instructions:
                si = inst.sync_info
                if si is not None and si.on_wait and len(si.on_wait) > max_waits:
                    waits = list(si.on_wait)
                    extra, keep = waits[:-max_waits], waits[-max_waits:]
                    for j, w in enumerate(extra):
                        new_insts.append(
                            mybir.InstDrain(
                                name=f"{inst.name}-presplit{j}",
                                engine=inst.engine,
                                sync_info=mybir.SyncInfo(on_wait=[w], on_update=[]),
                            )
                        )
                    si.on_wait = keep
                new_insts.append(inst)
            blk.instructions = new_insts


def build_bahdanau_nc(split_drains=True, reps=1, big_io=True, keys_mode="xbar_dram"):
    """Build the per-core Bass program (identical on all 8 cores)."""
    import contextlib

    nc = bass.Bass(trn_type="TRN2", target_bir_lowering=False, debug=False)

    big = "ExternalInput" if big_io else "Internal"
    # keys/values/wkey are pre-cast to bf16 on the host (make_in_maps):
    # halves HBM traffic, drops all DVE rounds, and enables the proven
    # DRAM->SBUF xbar DMA transpose for keysT
    keys_d = nc.dram_tensor("keys", [TB, T, H], bf16, kind=big).ap()
    values_d = nc.dram_tensor("values", [TB, T, H], bf16, kind=big).ap()
    wkey_d = nc.dram_tensor("wkey", [H, H], bf16, kind=big).ap()
    wquery_d = nc.dram_tensor("wquery", [H, H], f32, kind=big).ap()
    # queryt: host-prearranged query^T as [p, kchunk, b]
    qtin_d = nc.dram_tensor("qtin", [P, NC_, TB], f32, kind="ExternalInput").ap()
    # w_score host-prearranged as [p, kchunk]
    wsc_d = nc.dram_tensor("wsc", [P, NC_], f32, kind="ExternalInput").ap()
    # additive mask bias (0 where visible, -1e30 where masked)
    maskb_d = nc.dram_tensor("maskb", [TB, T], f32, kind="ExternalInput").ap()

    ctx_d = nc.dram_tensor("ctx", [TB, H], f32, kind="ExternalOutput").ap()
    alph_d = nc.dram_tensor("alph", [TB, T], f32, kind="ExternalOutput").ap()

    G = 4  # t/k-chunks per staging DMA

    with tile.TileContext(nc) as tc, contextlib.ExitStack() as ctx:
        # ---- pools
        const_pool = ctx.enter_context(tc.tile_pool(name="const", bufs=1))
        ktr_pool = ctx.enter_context(tc.tile_pool(name="ktr", bufs=3))
        s_pool = ctx.enter_context(tc.tile_pool(name="spool", bufs=2))
        v_pool = ctx.enter_context(tc.tile_pool(name="vpool", bufs=2))
        row_pool = ctx.enter_context(tc.tile_pool(name="rows", bufs=3))
        small_pool = ctx.enter_context(tc.tile_pool(name="small", bufs=2))
        knat_pool = ctx.enter_context(tc.tile_pool(name="knat", bufs=2))

        ps_pool = ctx.enter_context(tc.tile_pool(name="psS", bufs=2, space="PSUM"))
        sc_psum = ctx.enter_context(tc.tile_pool(name="scps", bufs=2, space="PSUM"))
        tl_psum = ctx.enter_context(tc.tile_pool(name="tlps", bufs=2, space="PSUM"))

        # ---- preamble
        ident = const_pool.tile([P, P], f32, tag="ident", name="ident")
        make_identity(nc, ident[:, :])
        identb = const_pool.tile([P, P], bf16, tag="identb", name="identb")
        nc.vector.tensor_copy(identb[:, :], ident[:, :])

        # prefetch ACT tables for Tanh/Exp during startup DMAs
        warm = const_pool.tile([1, 1], f32, tag="warm", name="warm")
        nc.scalar.activation(warm[:, :], ident[0:1, 0:1], ACT.Tanh)
        nc.scalar.activation(warm[:, :], ident[0:1, 0:1], ACT.Exp)

        # W_key: direct bf16 load into stationary tiles
        wk = const_pool.tile([P, NC_, H], bf16, tag="wk", name="wk")
        nc.sync.dma_start(
            wk[:, :, :], wkey_d[:, :].rearrange("(c p) h -> p c h", p=P)
        )

        qtin = const_pool.tile([P, NC_, TB], f32, tag="qtin", name="qtin")
        nc.sync.dma_start(qtin[:, :, :], qtin_d[:, :, :])
        wsc_raw = const_pool.tile([P, NC_], f32, tag="wsc_raw", name="wsc_raw")
        nc.sync.dma_start(wsc_raw[:, :], wsc_d[:, :])
        wsc = const_pool.tile([P, NC_], bf16, tag="wsc", name="wsc")
        nc.vector.tensor_copy(wsc[:, :], wsc_raw[:, :])
        # W_query: staged through the knat ring (dead after the q projection)
        wq4 = []
        for g in range(2):
            wqt = knat_pool.tile([P, G, H], f32, tag="knat", name=f"wq{g}")
            nc.sync.dma_start(
                wqt[:, :, :],
                wquery_d[g * G * P : (g + 1) * G * P, :].rearrange(
                    "(c p) h -> p c h", p=P
                ),
            )
            wq4.append(wqt)
        qT = const_pool.tile([P, NC_, TB], f32, tag="qT", name="qT")

        def emit_keys(b, ktr_dst):
            """keysT via 8 DRAM->SBUF xbar DMA transposes (contiguous source
            column-slab and contiguous [128,1024] destination -- the pattern
            tile_matmul.py uses on HW), or v1-style PE transposes in bf16."""
            if keys_mode == "xbar_dram":
                for k in range(NC_):
                    nc.sync.dma_start_transpose(
                        ktr_dst[:, k, :], keys_d[b, :, k * P : (k + 1) * P]
                    )
                return
            for g in range(2):
                knb = knat_pool.tile([P, G, H], bf16, tag="knb", name=f"knb{b}_{g}")
                nc.sync.dma_start(
                    knb[:, :, :],
                    keys_d[b, g * G * P : (g + 1) * G * P, :].rearrange(
                        "(c p) h -> p c h", p=P
                    ),
                )
                for j in range(G):
                    t = g * G + j
                    if True:
                        for h in range(2):
                            ptr = tl_psum.tile(
                                [P, 4 * P], bf16, tag="tl", name=f"ptr{b}_{t}_{h}"
                            )
                            for jj in range(4):
                                k = 4 * h + jj
                                nc.tensor.transpose(
                                    ptr[:, jj * P : (jj + 1) * P],
                                    knb[:, j, k * P : (k + 1) * P],
                                    identb[:, :],
                                )
                            src_ = ptr[:, :].rearrange("p (k c) -> p k c", k=4)
                            dst = ktr_dst[:, 4 * h : 4 * h + 4, t * P : (t + 1) * P]
                            if h == 0:
                                nc.vector.tensor_copy(dst, src_)
                            else:
                                nc.scalar.copy(dst, src_)

        # ---- steady-state batch pipeline (reps>1 repeats for timing only)
        for rep in range(reps):
            mb_cur = small_pool.tile([1, T], f32, tag="mb", name=f"mb_r{rep}b0")
            nc.sync.dma_start(mb_cur[:, :], maskb_d[0:1, :])
            ktrs = {0: ktr_pool.tile([P, NC_, T], bf16, tag="ktr", name=f"ktr_r{rep}b0")}
            emit_keys(0, ktrs[0])
            if TB > 1:
                ktrs[1] = ktr_pool.tile(
                    [P, NC_, T], bf16, tag="ktr", name=f"ktr_r{rep}b1"
                )
                emit_keys(1, ktrs[1])
            ktr_cur = ktrs[0]

            if rep == 0:
                # q projection (exact fp32); all 64 [h,b] columns accumulate
                # in one PSUM tile borrowed from the ps ring
                psq = ps_pool.tile([P, T], f32, tag="ps", name="psq")
                for m in range(NC_):
                    for k in range(NC_):
                        nc.tensor.matmul(
                            psq[:, m * TB : (m + 1) * TB],
                            lhsT=wq4[k // G][:, k % G, m * P : (m + 1) * P],
                            rhs=qtin[:, k, :],
                            start=(k == 0),
                            stop=(k == NC_ - 1),
                        )
                nc.scalar.copy(
                    qT[:, :, :],
                    psq[:, 0 : NC_ * TB].rearrange("p (m b) -> p m b", m=NC_),
                )

            pending = None  # batch whose PE tail (alpha transposes+ctx) is due

            def emit_tail(pb, arow_exp, rinv, vts):
                """PE tail of batch pb: alphas row->cols, ctx matmuls, drains."""
                paT = tl_psum.tile([P, TB], f32, tag="tl", name=f"paT{rep}_{pb}")
                for k in range(NC_):
                    nc.tensor.transpose(
                        paT[:, k : k + 1],
                        arow_exp[0:1, k * P : (k + 1) * P],
                        ident[0:1, 0:1],
                    )
                aT = small_pool.tile([P, NC_], bf16, tag="aT", name=f"aT{rep}_{pb}")
                nc.scalar.copy(aT[:, :], paT[:, :])

                pcx = [
                    tl_psum.tile([1, NH], f32, tag="tl", name=f"pcx{rep}_{pb}_{n}")
                    for n in range(2)
                ]
                for k in range(NC_):
                    for n in range(2):
                        nc.tensor.matmul(
                            pcx[n][:, :],
                            lhsT=aT[:, k : k + 1],
                            rhs=vts[k // G][:, k % G, n * NH : (n + 1) * NH],
                            start=(k == 0),
                            stop=(k == NC_ - 1),
                        )
                # drain with deferred softmax normalization: ctx = pcx * rinv
                cxr = row_pool.tile([1, T], f32, tag="row", name=f"cxr{rep}_{pb}")
                for n in range(2):
                    nc.scalar.mul(
                        cxr[:, n * NH : (n + 1) * NH], pcx[n][:, :], rinv[:, :]
                    )
                nc.sync.dma_start(ctx_d[pb : pb + 1, :], cxr[0:1, :H])

            for b in range(TB):
                last = b == TB - 1
                if not last:
                    mb_next = small_pool.tile(
                        [1, T], f32, tag="mb", name=f"mb_r{rep}b{b + 1}"
                    )
                    nc.sync.dma_start(mb_next[:, :], maskb_d[b + 1 : b + 2, :])
                    if b + 2 < TB:
                        ktrs[b + 2] = ktr_pool.tile(
                            [P, NC_, T], bf16, tag="ktr", name=f"ktr_r{rep}b{b + 2}"
                        )
                        emit_keys(b + 2, ktrs[b + 2])

                # values prefetch: 2 direct bf16 DMAs (consumed by the
                # ctx matmul one batch later -- bufs=4 keeps 2 batches)
                vts = []
                for g in range(2):
                    vt = v_pool.tile(
                        [P, G, H], bf16, tag="v", bufs=4, name=f"v{rep}_{b}_{g}"
                    )
                    nc.sync.dma_start(
                        vt[:, :, :],
                        values_d[b, g * G * P : (g + 1) * G * P, :].rearrange(
                            "(c p) h -> p c h", p=P
                        ),
                    )
                    vts.append(vt)

                # main matmul + tanh + scores
                psc = [
                    sc_psum.tile([1, NH], f32, tag="sm", name=f"psc{rep}_{b}_{n}")
                    for n in range(2)
                ]
                for m in range(NC_):
                    ps = ps_pool.tile([P, T], f32, tag="ps", name=f"ps{rep}_{b}_{m}")
                    for k in range(NC_):
                        for n in range(2):
                            nc.tensor.matmul(
                                ps[:, n * NH : (n + 1) * NH],
                                lhsT=wk[:, k, m * P : (m + 1) * P],
                                rhs=ktr_cur[:, k, n * NH : (n + 1) * NH],
                                start=(k == 0),
                                stop=(k == NC_ - 1),
                            )
                    s = s_pool.tile([P, T], bf16, tag="s", name=f"s{rep}_{b}_{m}")
                    nc.scalar.activation(
                        s[:, :], ps[:, :], ACT.Tanh, bias=qT[:, m, b : b + 1]
                    )
                    for n in range(2):
                        nc.tensor.matmul(
                            psc[n][:, :],
                            lhsT=wsc[:, m : m + 1],
                            rhs=s[:, n * NH : (n + 1) * NH],
                            start=(m == 0),
                            stop=(m == NC_ - 1),
                        )
                    if m == TAIL_M and pending is not None:
                        emit_tail(*pending)
                        pending = None

                # softmax head (ACT/DVE only; no max-subtraction -- scores
                # are bounded by |w_score|_1 ~ 26, exp is safe in fp32)
                sc = row_pool.tile([1, T], f32, tag="row", name=f"sc{rep}_{b}")
                for n in range(2):
                    nc.vector.tensor_add(
                        sc[:, n * NH : (n + 1) * NH],
                        psc[n][:, :],
                        mb_cur[:, n * NH : (n + 1) * NH],
                    )
                arow_exp = row_pool.tile([1, T], f32, tag="row", name=f"ae{rep}_{b}")
                ssum = small_pool.tile([1, 1], f32, tag="ssum", name=f"ssum{rep}_{b}")
                nc.scalar.activation(
                    arow_exp[:, :], sc[:, :], ACT.Exp, accum_out=ssum[:, :]
                )
                rinv = small_pool.tile([1, 1], f32, tag="rinv", name=f"rinv{rep}_{b}")
                nc.vector.reciprocal(rinv[:, :], ssum[:, :])
                # normalized alphas row out (off critical path)
                arow_n = row_pool.tile([1, T], f32, tag="rowo", name=f"an{rep}_{b}")
                nc.vector.tensor_scalar_mul(arow_n[:, :], arow_exp[:, :], rinv[:, :])
                nc.sync.dma_start(alph_d[b : b + 1, :], arow_n[:, :])

                pending = (b, arow_exp, rinv, vts)
                if not last:
                    ktr_cur = ktrs[b + 1]
                    mb_cur = mb_next
                del ktrs[b]
            emit_tail(*pending)

    if split_drains:
        _split_drain_waits(nc)
    return nc


_NC_CACHE = None


def _get_nc():
    global _NC_CACHE
    if _NC_CACHE is None:
        _NC_CACHE = build_bahdanau_nc()
    return _NC_CACHE


def make_in_maps(query, mask, values, keys, W_key, W_query, w_score):
    """Shard full inputs into per-core input maps (host-side layout +
    bf16 pre-cast of the large tensors)."""
    import ml_dtypes

    bf = ml_dtypes.bfloat16
    query = np.ascontiguousarray(np.asarray(query, dtype=np.float32))
    mask = np.asarray(mask)
    values = np.ascontiguousarray(np.asarray(values, dtype=np.float32).astype(bf))
    keys = np.ascontiguousarray(np.asarray(keys, dtype=np.float32).astype(bf))
    W_key = np.ascontiguousarray(np.asarray(W_key, dtype=np.float32).astype(bf))
    W_query = np.ascontiguousarray(np.asarray(W_query, dtype=np.float32))
    w_score = np.ascontiguousarray(np.asarray(w_score, dtype=np.float32))

    B = query.shape[0]
    n_cores = B // TB
    maskb = np.where(mask, np.float32(0.0), np.float32(-1e30)).astype(np.float32)
    wsc_in = np.ascontiguousarray(w_score.reshape(NC_, P).T)

    in_maps = []
    for c in range(n_cores):
        sl = slice(c * TB, (c + 1) * TB)
        qt = query[sl, 0, :].T  # [QS, TB]
        qtin = np.ascontiguousarray(qt.reshape(NC_, P, TB).transpose(1, 0, 2))
        in_maps.append(
            {
                "keys": keys[sl],
                "values": values[sl],
                "wkey": W_key,
                "wquery": W_query,
                "qtin": qtin,
                "wsc": wsc_in,
                "maskb": np.ascontiguousarray(maskb[sl]),
            }
        )
    return in_maps


def kernel(query, mask, values, keys, W_key, W_query, w_score):
    from concourse.bass_utils import run_bass_kernel_spmd

    B = np.asarray(query).shape[0]
    n_cores = B // TB
    in_maps = make_in_maps(query, mask, values, keys, W_key, W_query, w_score)
    nc = _get_nc()
    try:
        res = run_bass_kernel_spmd(nc, in_maps, core_ids=list(range(n_cores)))
    except Exception:
        # transient NRT_EXEC_UNIT_UNRECOVERABLE wedges have been observed to
        # clear on retry
        import time as _time

        _time.sleep(2.0)
        res = run_bass_kernel_spmd(nc, in_maps, core_ids=list(range(n_cores)))
    context = np.concatenate([r["ctx"] for r in res.results], axis=0)
    alphas = np.concatenate([r["alph"] for r in res.results], axis=0)
    return context.reshape(B, 1, H), alphas.reshape(B, 1, T)

